# revision 1
# baseline (speedup 1.0000x reference)
"""Multi-Head Latent Attention (MLA) TRN2 Bass kernel.

Sharding: data-parallel over batch (B=2) x tensor-parallel over heads
(16 heads -> 4 per core) = 8 cores. The kv_lora latent path and shared
rope key are computed replicated within each batch group (cross-core
AllGather / Pool-engine offload both trip the chip power throttle and
net out slower); the final output projection is computed as per-core
partials which the host sums.

All on-device dataflow is "transposed" (feature dim on partitions,
sequence on the free dim) so no PE transposes are ever needed:
  qT      = Wq_perm^T @ xT          [768, S]   (nope tiles written to fp8)
  kv_aT   = Wkv_a^T @ xT            [576, S]   (c_kvT + k_ropeT)
  k_nopeT = Wkv_b_k^T @ c_kvT       [512, S]   (written to fp8)
  v       = (c_kvT chunk)^T-matmuls [S, 512]   (natural layout)
  RoPE applied in transposed layout with a partition-swap DMA + 3 DVE ops
  scoresT[s_k, s_q] per (head, q-block of 512) via a single fp8e4
  DoubleRow matmul per chunk (the 192-dim nope+rope contraction packed as
  2x96 partitions, q/k pre-scaled by 64/32 and descaled inside the exp),
  causal masks added on the 4 diagonal chunks by a bf16 PE matmul,
  exp on ACT (no max subtraction; scores are bounded), softmax
  denominators via running elementwise bf16 sums of the exp'd chunks on
  the DVE + a single ones-matmul per (head, q-block), outT accumulated
  in PSUM, normalized by broadcasted reciprocals, then
  partialT = Wo_c^T @ outT, staged per [128,512] block and DMA'd out.

x and outp use chunk-major DRAM layouts so every DMA is one dense
contiguous block (host packs/unpacks).
"""

import math
import sys

import numpy as np
import ml_dtypes

try:  # concourse ships in the container; fall back to the repo checkout
    import concourse.bass  # noqa: F401
except ImportError:  # pragma: no cover
    for p in ("/opt/trn_rl_repo", "/root/.axon_site/_ro/trn_rl_repo"):
        if p not in sys.path:
            sys.path.insert(0, p)

# Problem constants (hardcoded; harness calls kernel() standalone).
D_MODEL = 2048
N_HEADS = 16
R = 512          # kv lora rank
DN = 128         # d_nope
DR = 64          # d_rope
DV = 128         # d_v
ROPE_THETA = 10000.0
B = 2
S = 2048
HP = 4           # heads per core
QB = 512         # q block size
NKC = S // 128   # 16 k chunks
NQB = S // QB    # 4 q blocks
NCORES = 8

BF16 = ml_dtypes.bfloat16

# Experiment toggles (final best config ships as the defaults).
# The AllGather path and Pool-engine accumulation both trip the chip's
# power throttle (PE capped to 50% util for 25-50% of the run) and lose
# more than they save -- keep both OFF.
USE_CC = False    # shard c_kv down-projection + AllGather vs replicate
POOL_ACC = False  # denominator chunk-sums on Pool for odd heads vs DVE only
FP8_SCORES = True  # fp8e4 DoubleRow score matmuls (K=192 packed as 2x96)

_PROGRAM = {}


def _build_program(split_waits=True):
    use_cc = USE_CC
    pool_acc = POOL_ACC
    fp8_scores = FP8_SCORES
    import concourse.bass as bass
    import concourse.mybir as mybir
    from concourse.tile import TileContext

    def split_multi_waits(max_waits=1):
        """The walrus build in this container rejects instructions with
        more than `max_waits` sync-wait commands. Move excess waits onto
        same-engine NoOps inserted just before the instruction."""
        for f in nc.m.functions:
            for bb in f.blocks:
                out = []
                changed = False
                for inst in bb.instructions:
                    si = getattr(inst, "sync_info", None)
                    ws = list(si.on_wait) if si is not None else []
                    if len(ws) > max_waits:
                        changed = True
                        inst.sync_info = mybir.SyncInfo(
                            on_wait=ws[:max_waits],
                            on_update=list(si.on_update))
                        for w in ws[max_waits:]:
                            n = mybir.InstNoOp(
                                name=nc.get_next_instruction_name(),
                                ins=[], outs=[])
                            n.engine = inst.engine
                            n.sync_info = mybir.SyncInfo(
                                on_wait=[w], on_update=[])
                            out.append(n)
                    out.append(inst)
                if changed:
                    bb.instructions = out

    f32 = mybir.dt.float32
    cdt = mybir.dt.bfloat16
    f8 = mybir.dt.float8e4
    DRmode = mybir.MatmulPerfMode.DoubleRow
    Copy = mybir.ActivationFunctionType.Copy
    # fp8 pre-scales for q/k (folded back out inside the exp)
    QS, KS = 64.0, 32.0

    nc = bass.Bass()

    # x arrives chunk-major: block (t, k) = xT_logical[128k:128k+128,
    # 512t:512t+512] stored contiguously so every chunk DMA is one dense
    # 128KB transfer instead of 128 separate 1KB rows.
    xT = nc.dram_tensor("xT", [NQB * 16 * 128, QB], cdt, kind="ExternalInput")
    wq = nc.dram_tensor("wq", [D_MODEL, HP * (DN + DR)], cdt, kind="ExternalInput")
    # use_cc: cols 0:128 = this core's c_kv latent row shard, 128:192 = rope
    # else:   cols 0:512 = full c_kv latent, 512:576 = rope
    wkva_cols = (128 + DR) if use_cc else (R + DR)
    wkva = nc.dram_tensor("wkva", [D_MODEL, wkva_cols], cdt, kind="ExternalInput")
    wkvbk = nc.dram_tensor("wkvbk", [R, HP * DN], cdt, kind="ExternalInput")
    wkvbv = nc.dram_tensor("wkvbv", [R, HP * DV], cdt, kind="ExternalInput")
    wo = nc.dram_tensor("wo", [HP * DV, D_MODEL], cdt, kind="ExternalInput")
    cosf = nc.dram_tensor("cosf", [128, S], cdt, kind="ExternalInput")
    sinf = nc.dram_tensor("sinf", [128, S], cdt, kind="ExternalInput")
    masks = nc.dram_tensor("masks", [128, 128], cdt, kind="ExternalInput")
    ident = nc.dram_tensor("ident", [128, 128], cdt, kind="ExternalInput")
    ones = nc.dram_tensor("ones", [128, 1], cdt, kind="ExternalInput")
    onesf = nc.dram_tensor("onesf", [1, 128], cdt, kind="ExternalInput")
    # outp is block-major: block (m, nb) stored contiguously; host unpacks.
    outp = nc.dram_tensor("outp", [16 * NQB * 128, QB], f32, kind="ExternalOutput")

    Exp = mybir.ActivationFunctionType.Exp
    Ln = mybir.ActivationFunctionType.Ln

    with TileContext(nc) as tc:
        with (
            tc.tile_pool(name="const", bufs=1) as cpool,
            tc.tile_pool(name="persist", bufs=1) as ppool,
        ):
            cosf_sb = cpool.tile([128, S], cdt, name="cosf_sb")
            sinf_sb = cpool.tile([128, S], cdt, name="sinf_sb")
            masks_sb = cpool.tile([128, 128], cdt, name="masks_sb")
            ident_sb = cpool.tile([128, 128], cdt, name="ident_sb")
            ones_sb = cpool.tile([128, 1], cdt, name="ones_sb")
            onesb_sb = cpool.tile([1, 128], cdt, name="onesb_sb")

            # Persistent activations.
            if fp8_scores:
                # q_nope / k_nope tiles live in fp8 (written pre-scaled
                # straight from PSUM); rope halves stay bf16 until after
                # the RoPE rotation, then are cast.
                qT8 = [
                    ppool.tile([128, S], f8, name=f"qT8_{m}", tag="qT8",
                               bufs=4)
                    for m in range(4)
                ]
                qTr = [
                    ppool.tile([128, S], cdt, name=f"qTr{m}", tag="qT",
                               bufs=2)
                    for m in range(2)
                ]
                qT = qT8 + qTr  # indexable like the bf16 path
                kn8 = [
                    ppool.tile([128, S], f8, name=f"kn8_{m}", tag="kn8",
                               bufs=4)
                    for m in range(4)
                ]
                qr8 = [
                    ppool.tile([128, S], f8, name=f"qr8_{m}", tag="qr8",
                               bufs=2)
                    for m in range(2)
                ]
                kr8 = ppool.tile([128, S], f8, name="kr8", tag="kr8", bufs=1)
                # DoubleRow-packed per-head tensors: 192 contraction dims
                # as 2 blocks of 96 partitions (blk0 = dims 0:96,
                # blk1 = dims 96:192 = nope 96:128 + rope 0:64).
                qf8 = [
                    ppool.tile([96, NQB, 2, QB], f8, name=f"qf8_{h}",
                               tag="qf8", bufs=4)
                    for h in range(HP)
                ]
                kf8 = [
                    ppool.tile([96, NKC, 2, 128], f8, name=f"kf8_{h}",
                               tag="kf8", bufs=4)
                    for h in range(HP)
                ]
            else:
                qT = [
                    ppool.tile([128, S], cdt, name=f"qT{m}", tag="qT", bufs=6)
                    for m in range(6)
                ]
                kn = [
                    ppool.tile([128, S], cdt, name=f"kn{m}", tag="kn", bufs=4)
                    for m in range(4)
                ]
            ck = [
                ppool.tile([128, S], cdt, name=f"ck{m}", tag="cko", bufs=4)
                for m in range(4)
            ]
            kr = ppool.tile([128, S], cdt, name="krope", tag="krope", bufs=1)

            def store_q(m, cols, ps):
                # PSUM -> SBUF store for qT M-tile m (fp8 path pre-scales
                # the nope tiles; rope tiles stay bf16 for RoPE).
                if fp8_scores and m < 4:
                    nc.scalar.activation(qT8[m][:, cols], ps, Copy, scale=QS)
                else:
                    nc.scalar.copy(qT[m][:, cols], ps)

            def store_kn(m, cols, ps):
                if fp8_scores:
                    nc.scalar.activation(kn8[m][:, cols], ps, Copy, scale=KS)
                else:
                    nc.scalar.copy(kn[m][:, cols], ps)
            vt = [
                ppool.tile([128, HP * DV], cdt, name=f"v{i}", tag="v", bufs=NKC)
                for i in range(NKC)
            ]
            # RoPE swap scratch lives in the persistent pool so the kvT
            # weight pool does not WAR-serialize against the rope phase.
            swt = [
                ppool.tile([128, S], cdt, name=f"sw{i}", tag="sw", bufs=3)
                for i in range(3)
            ]
            # Local c_kv shard staging (bf16) feeding the AllGather bounce.
            csh = [
                ppool.tile([128, QB], cdt, name=f"csh{t}", tag="csh", bufs=4)
                for t in range(NQB)
            ]

            # kv_b weights persist so their DMAs can issue at startup.
            wbk_sb = [
                ppool.tile([128, HP * DN], cdt, name=f"wbk_sb{r}", tag="wbk",
                           bufs=4)
                for r in range(4)
            ]
            wbv_sb = [
                ppool.tile([128, HP * DV], cdt, name=f"wbv_sb{r}", tag="wbv",
                           bufs=4)
                for r in range(4)
            ]

            # ---- Phase 1: x projections, AllGather, kv up-projection ----
            with (
                tc.tile_pool(name="wproj", bufs=1) as wpool,
                tc.tile_pool(name="xstream", bufs=1) as xpool,
                tc.tile_pool(name="psA", bufs=8, space="PSUM") as psA,
                tc.tile_pool(name="dram", bufs=1, space="DRAM") as dram,
            ):
                if use_cc:
                    bounce_in = dram.tile([128, S], cdt, name="bounce_in")
                    bounce_out = dram.tile([R, S], cdt, name="bounce_out")

                # Interleave weight-chunk and first-quarter x DMAs across two
                # HWDGE queues so the first matmuls can start early.
                wq_sb = []
                wkva_sb = []
                xq0 = []
                for k in range(16):
                    xk = xpool.tile([128, QB], cdt, name=f"xq_0_{k}",
                                    tag="xq", bufs=24)
                    nc.sync.dma_start(xk, xT[k * 128:(k + 1) * 128, :])
                    xq0.append(xk)
                    # wq first: the very first PE matmul (qT m0) needs it;
                    # the wkva weights are only consumed 6 matmuls later.
                    w1 = wpool.tile([128, HP * (DN + DR)], cdt, name=f"wq_sb{k}",
                                    tag="wq", bufs=16)
                    nc.scalar.dma_start(w1, wq[k * 128:(k + 1) * 128, :])
                    wq_sb.append(w1)
                    w2 = wpool.tile([128, wkva_cols], cdt, name=f"wkva_sb{k}",
                                    tag="wkva", bufs=16)
                    nc.scalar.dma_start(w2, wkva[k * 128:(k + 1) * 128, :])
                    wkva_sb.append(w2)
                for r in range(4):
                    nc.scalar.dma_start(wbk_sb[r], wkvbk[r * 128:(r + 1) * 128, :])
                    nc.scalar.dma_start(wbv_sb[r], wkvbv[r * 128:(r + 1) * 128, :])
                nc.scalar.dma_start(cosf_sb, cosf[:, :])
                nc.scalar.dma_start(sinf_sb, sinf[:, :])
                nc.scalar.dma_start(masks_sb, masks[:, :])
                nc.scalar.dma_start(ident_sb, ident[:, :])
                nc.scalar.dma_start(ones_sb, ones[:, :])
                nc.scalar.dma_start(onesb_sb, onesf[:, :])

                # Quarter 0 is DMA-latency bound: run k OUTER over all 8
                # accumulators so each arriving x chunk gets 8 matmuls of
                # work instead of 1.
                rope_col = 128 if use_cc else R
                t0 = slice(0, QB)
                if use_cc:
                    ps_sh = psA.tile([128, QB], f32, name="ps_sh0", tag="ps")
                    ps_r = psA.tile([64, QB], f32, name="psr_0", tag="ps")
                    ps_q = [
                        psA.tile([128, QB], f32, name=f"psq0_{m}", tag="ps")
                        for m in range(6)
                    ]
                    for k in range(16):
                        nc.tensor.matmul(
                            ps_sh, lhsT=wkva_sb[k][:, 0:128],
                            rhs=xq0[k], start=(k == 0), stop=(k == 15))
                        nc.tensor.matmul(
                            ps_r, lhsT=wkva_sb[k][:, rope_col:rope_col + DR],
                            rhs=xq0[k], start=(k == 0), stop=(k == 15))
                        for m in range(6):
                            nc.tensor.matmul(
                                ps_q[m], lhsT=wq_sb[k][:, m * 128:(m + 1) * 128],
                                rhs=xq0[k], start=(k == 0), stop=(k == 15))
                    nc.vector.tensor_copy(csh[0], ps_sh)
                    # Software-DGE queue: keeps the HWDGE x-stream free of
                    # head-of-line blocking, and orders naturally before the
                    # collective trigger on the same engine.
                    nc.gpsimd.dma_start(bounce_in[:, t0], csh[0])
                    nc.scalar.copy(kr[0:64, t0], ps_r)
                    nc.scalar.copy(kr[64:128, t0], ps_r)
                    for m in range(6):
                        store_q(m, t0, ps_q[m])
                else:
                    ps8 = [
                        psA.tile([128, QB], f32, name=f"psq0_{m}", tag="ps")
                        for m in range(8)
                    ]
                    for k in range(16):
                        for m in range(6):
                            nc.tensor.matmul(
                                ps8[m], lhsT=wq_sb[k][:, m * 128:(m + 1) * 128],
                                rhs=xq0[k], start=(k == 0), stop=(k == 15))
                        for m in range(2):
                            nc.tensor.matmul(
                                ps8[6 + m],
                                lhsT=wkva_sb[k][:, m * 128:(m + 1) * 128],
                                rhs=xq0[k], start=(k == 0), stop=(k == 15))
                    for m in range(6):
                        store_q(m, t0, ps8[m])
                    for m in range(2):
                        nc.vector.tensor_copy(ck[m][:, t0], ps8[6 + m])
                    for m in (2, 3):
                        ps = psA.tile([128, QB], f32, name=f"psk_0_{m}", tag="ps")
                        for k in range(16):
                            nc.tensor.matmul(
                                ps, lhsT=wkva_sb[k][:, m * 128:(m + 1) * 128],
                                rhs=xq0[k], start=(k == 0), stop=(k == 15))
                        nc.vector.tensor_copy(ck[m][:, t0], ps)
                    ps = psA.tile([64, QB], f32, name="psr_0", tag="ps")
                    for k in range(16):
                        nc.tensor.matmul(
                            ps, lhsT=wkva_sb[k][:, R:R + DR],
                            rhs=xq0[k], start=(k == 0), stop=(k == 15))
                    nc.scalar.copy(kr[0:64, t0], ps)
                    nc.scalar.copy(kr[64:128, t0], ps)

                for t in range(1, NQB):
                    tcols = slice(t * QB, (t + 1) * QB)
                    xq = []
                    for k in range(16):
                        xk = xpool.tile([128, QB], cdt, name=f"xq_{t}_{k}",
                                        tag="xq", bufs=24)
                        nc.sync.dma_start(
                            xk, xT[(t * 16 + k) * 128:(t * 16 + k + 1) * 128, :])
                        xq.append(xk)
                    if use_cc:
                        # c_kv shard first so the AllGather can start as
                        # early as possible once quarter 3's shard lands.
                        ps = psA.tile([128, QB], f32, name=f"ps_sh{t}",
                                      tag="ps")
                        for k in range(16):
                            nc.tensor.matmul(
                                ps, lhsT=wkva_sb[k][:, 0:128],
                                rhs=xq[k], start=(k == 0), stop=(k == 15))
                        nc.vector.tensor_copy(csh[t], ps)
                        nc.gpsimd.dma_start(bounce_in[:, tcols], csh[t])
                    else:
                        for m in range(4):
                            ps = psA.tile([128, QB], f32, name=f"psk_{t}_{m}",
                                          tag="ps")
                            for k in range(16):
                                nc.tensor.matmul(
                                    ps,
                                    lhsT=wkva_sb[k][:, m * 128:(m + 1) * 128],
                                    rhs=xq[k], start=(k == 0), stop=(k == 15))
                            nc.vector.tensor_copy(ck[m][:, tcols], ps)
                    # k_ropeT, duplicated into kr
                    ps = psA.tile([64, QB], f32, name=f"psr_{t}", tag="ps")
                    for k in range(16):
                        nc.tensor.matmul(
                            ps, lhsT=wkva_sb[k][:, rope_col:rope_col + DR],
                            rhs=xq[k], start=(k == 0), stop=(k == 15))
                    nc.scalar.copy(kr[0:64, tcols], ps)
                    nc.scalar.copy(kr[64:128, tcols], ps)
                    # qT M-tiles
                    for m in range(6):
                        ps = psA.tile([128, QB], f32, name=f"psq_{t}_{m}", tag="ps")
                        for k in range(16):
                            nc.tensor.matmul(
                                ps, lhsT=wq_sb[k][:, m * 128:(m + 1) * 128],
                                rhs=xq[k], start=(k == 0), stop=(k == 15))
                        store_q(m, tcols, ps)

                if use_cc:
                    # Exchange c_kv shards within the batch group while the
                    # PE grinds through the qT tiles above.
                    nc.gpsimd.collective_compute(
                        "AllGather",
                        mybir.AluOpType.bypass,
                        replica_groups=[[0, 1, 2, 3], [4, 5, 6, 7]],
                        ins=[bounce_in[:]],
                        outs=[bounce_out[:]],
                    )
                    for r in range(4):
                        nc.scalar.dma_start(
                            ck[r], bounce_out[r * 128:(r + 1) * 128, :])

                # ---- kv up-projection (k_nopeT, v); same PSUM pool ----
                for m in range(4):
                    for nb in range(NQB):
                        ncols = slice(nb * QB, (nb + 1) * QB)
                        ps = psA.tile([128, QB], f32, name=f"psn_{m}_{nb}",
                                      tag="ps")
                        for r in range(4):
                            nc.tensor.matmul(
                                ps, lhsT=wbk_sb[r][:, m * 128:(m + 1) * 128],
                                rhs=ck[r][:, ncols], start=(r == 0),
                                stop=(r == 3))
                        store_kn(m, ncols, ps)
                for i in range(NKC):
                    ps = psA.tile([128, HP * DV], f32, name=f"psv_{i}", tag="ps")
                    for r in range(4):
                        nc.tensor.matmul(
                            ps, lhsT=ck[r][:, i * 128:(i + 1) * 128],
                            rhs=wbv_sb[r], start=(r == 0), stop=(r == 3))
                    nc.vector.tensor_copy(vt[i], ps)

            # ---- Phase 2: RoPE rotation (in place; overlaps on DVE) ----
            # rot = x * cosf + swap32(x) * sinf, where swap32 swaps each
            # 32-row half within every 64-row group (signs folded in sinf).
            for idx, tap in enumerate([qT[4], qT[5], kr]):
                sw = swt[idx]
                for blk in range(4):
                    src = (blk ^ 1) * 32
                    nc.sync.dma_start(
                        sw[blk * 32:(blk + 1) * 32, :],
                        tap[src:src + 32, :])
                nc.vector.tensor_mul(tap, tap, cosf_sb)
                nc.vector.tensor_mul(sw, sw, sinf_sb)
                nc.vector.tensor_add(tap, tap, sw)

            if fp8_scores:
                # Cast the roped halves to fp8 and assemble the per-head
                # DoubleRow-packed q/k tensors with partition-moving DMAs.
                nc.scalar.activation(qr8[0], qT[4], Copy, scale=QS)
                nc.scalar.activation(qr8[1], qT[5], Copy, scale=QS)
                nc.scalar.activation(kr8, kr, Copy, scale=KS)
                for h in range(HP):
                    off = (h % 2) * 64
                    ri = h // 2
                    nc.sync.dma_start(qf8[h][0:96, :, 0, :], qT8[h][0:96, :])
                    nc.sync.dma_start(qf8[h][0:32, :, 1, :], qT8[h][96:128, :])
                    nc.sync.dma_start(qf8[h][32:96, :, 1, :],
                                      qr8[ri][off:off + 64, :])
                    nc.scalar.dma_start(kf8[h][0:96, :, 0, :], kn8[h][0:96, :])
                    nc.scalar.dma_start(kf8[h][0:32, :, 1, :],
                                        kn8[h][96:128, :])
                    nc.scalar.dma_start(kf8[h][32:96, :, 1, :],
                                        kr8[off:off + 64, :])

            # outT tiles reuse the c_kvT slots (same tag, 4 bufs).
            outT = [
                ppool.tile([128, S], cdt, name=f"outT{h}", tag="cko", bufs=4)
                for h in range(HP)
            ]

            # ---- Phase 3: attention + output projection ----
            with (
                tc.tile_pool(name="att", bufs=1) as apool,
                tc.tile_pool(name="psS", bufs=4, space="PSUM") as psS,
                tc.tile_pool(name="psO", bufs=2, space="PSUM") as psO,
                tc.tile_pool(name="psD", bufs=1, space="PSUM") as psD,
                tc.tile_pool(name="psBC", bufs=1, space="PSUM") as psBC,
            ):
                # Wo loads overlap the attention phase on the idle sync queue.
                wo_sb = [
                    apool.tile([128, D_MODEL], cdt, name=f"wo_sb{r}", tag="wo",
                               bufs=4)
                    for r in range(4)
                ]
                for r in range(4):
                    nc.sync.dma_start(wo_sb[r], wo[r * 128:(r + 1) * 128, :])

                def calc_dps(h, j, ptacc):
                    # Single partition-sum matmul over the accumulated
                    # exp'd chunks (deferred off the critical path).
                    dps = psD.tile([1, QB], f32, name=f"dps_{h}_{j}", tag="d")
                    nc.tensor.matmul(dps, lhsT=ones_sb, rhs=ptacc,
                                     start=True, stop=True)
                    return dps

                def norm_early(h, j, dps):
                    # 1/denom as exp(-ln(d)) on the ACT engine. Moving this
                    # (or the bc copy below) to the DVE trips the power
                    # throttle and nets out slower -- measured.
                    rec = apool.tile([1, QB], f32, name=f"rec_{h}_{j}",
                                     tag="rec", bufs=2)
                    nc.scalar.activation(rec, dps, Ln)
                    recb = apool.tile([1, QB], cdt, name=f"recb_{h}_{j}",
                                      tag="recb", bufs=2)
                    nc.scalar.activation(recb, rec, Exp, scale=-1.0)
                    return recb

                def norm_late(h, j, ops, recb):
                    # Broadcast 1/denom across partitions via a K=1 matmul,
                    # then scale the out accumulator into outT.
                    qs = slice(j * QB, (j + 1) * QB)
                    bps = psBC.tile([128, QB], f32, name=f"bps_{h}_{j}",
                                    tag="b")
                    nc.tensor.matmul(bps, lhsT=onesb_sb, rhs=recb,
                                     start=True, stop=True)
                    bc = apool.tile([128, QB], f32, name=f"bc_{h}_{j}",
                                    tag="bc", bufs=2)
                    nc.scalar.copy(bc, bps)
                    nc.vector.tensor_mul(outT[h][:, qs], ops, bc)

                # Deferred normalization pipeline: unit (h,j)'s denominator
                # matmul runs during the NEXT unit's chunk 1 (so the PE
                # never stalls on the accumulation engine), and its
                # normalize runs during the next unit's chunk 3.
                pend = None  # (h, j, ops, ptacc) -> (h, j, ops, recb, True)
                for h in range(HP):
                    qn = qT[h]
                    qr = qT[4 + h // 2]
                    off = (h % 2) * 64
                    # Alternate accumulation engine so neither DVE nor Pool
                    # becomes the attention-phase straggler.
                    acc_eng = nc.vector if (h % 2 == 0 or not pool_acc) \
                        else nc.gpsimd
                    for j in range(NQB):
                        qs = slice(j * QB, (j + 1) * QB)
                        ops = psO.tile([128, QB], f32, name=f"ops_{h}_{j}",
                                       tag="o")
                        ptacc = apool.tile([128, QB], cdt,
                                           name=f"pta_{h}_{j}", tag="pta",
                                           bufs=2)
                        nch = 4 * (j + 1)
                        if fp8_scores:
                            # Chunk PAIRS share one 2-bank PSUM tile so a
                            # single exp covers both chunks -- the ACT
                            # engine is the attention bottleneck and each
                            # activation carries ~280ns fixed overhead.
                            for p in range(nch // 2):
                                cA, cB = 2 * p, 2 * p + 1
                                rA, rB = cA - 4 * j, cB - 4 * j
                                colA = max(0, rA * 128)
                                colB = max(0, rB * 128)
                                wA = slice(colA, QB)
                                wB = slice(colB, QB)
                                pairps = psS.tile(
                                    [128, 2 * QB], f32,
                                    name=f"sps_{h}_{j}_{p}", tag="s",
                                    bufs=2)
                                nc.tensor.matmul(
                                    pairps[:, colA:QB],
                                    lhsT=kf8[h][:, cA, :, :],
                                    rhs=qf8[h][:, j, :, colA:QB],
                                    start=True, stop=(rA < 0),
                                    perf_mode=DRmode, skip_group_check=True)
                                if rA >= 0:
                                    nc.tensor.matmul(
                                        pairps[:, colA:colA + 128],
                                        lhsT=ident_sb, rhs=masks_sb,
                                        start=False, stop=True,
                                        skip_group_check=True)
                                nc.tensor.matmul(
                                    pairps[:, QB + colB:2 * QB],
                                    lhsT=kf8[h][:, cB, :, :],
                                    rhs=qf8[h][:, j, :, colB:QB],
                                    start=True, stop=(rB < 0),
                                    perf_mode=DRmode, skip_group_check=True)
                                if rB >= 0:
                                    nc.tensor.matmul(
                                        pairps[:, QB + colB:QB + colB + 128],
                                        lhsT=ident_sb, rhs=masks_sb,
                                        start=False, stop=True,
                                        skip_group_check=True)
                                ptp = apool.tile([128, 2 * QB], cdt,
                                                 name=f"pt_{h}_{j}_{p}",
                                                 tag="pt", bufs=2)
                                nc.scalar.activation(
                                    ptp[:, colA:2 * QB],
                                    pairps[:, colA:2 * QB], Exp,
                                    scale=1.0 / (QS * KS))
                                nc.tensor.matmul(
                                    ops[:, wA],
                                    lhsT=vt[cA][:, h * DV:(h + 1) * DV],
                                    rhs=ptp[:, colA:QB], start=(cA == 0),
                                    stop=False, skip_group_check=True)
                                nc.tensor.matmul(
                                    ops[:, wB],
                                    lhsT=vt[cB][:, h * DV:(h + 1) * DV],
                                    rhs=ptp[:, QB + colB:2 * QB],
                                    start=False, stop=(cB == nch - 1),
                                    skip_group_check=True)
                                if cA == 0:
                                    acc_eng.tensor_copy(ptacc, ptp[:, 0:QB])
                                else:
                                    acc_eng.tensor_add(
                                        ptacc[:, wA], ptacc[:, wA],
                                        ptp[:, colA:QB])
                                acc_eng.tensor_add(
                                    ptacc[:, wB], ptacc[:, wB],
                                    ptp[:, QB + colB:2 * QB])
                                if p == 0 and pend is not None \
                                        and len(pend) == 4:
                                    ph, pj, pops, pacc = pend
                                    recb = norm_early(
                                        ph, pj, calc_dps(ph, pj, pacc))
                                    pend = (ph, pj, pops, recb, True)
                                if p == 1 and pend is not None \
                                        and len(pend) == 5:
                                    ph, pj, pops, recb, _ = pend
                                    norm_late(ph, pj, pops, recb)
                                    pend = None
                            pend = (h, j, ops, ptacc)
                            continue
                        for c in range(nch):
                            ks = slice(c * 128, (c + 1) * 128)
                            r = c - 4 * j
                            # Diagonal chunks only need columns >= r*128
                            # (everything to the left is strictly above the
                            # causal boundary). Chunk 0 always start-covers
                            # the full accumulator width.
                            col0 = max(0, r * 128)
                            w = slice(col0, QB)
                            qsw = slice(j * QB + col0, (j + 1) * QB)
                            sps = psS.tile([128, QB], f32,
                                           name=f"sps_{h}_{j}_{c}", tag="s")
                            if True:
                                nc.tensor.matmul(sps[:, w], lhsT=kn[h][:, ks],
                                                 rhs=qn[:, qsw], start=True,
                                                 stop=False,
                                                 skip_group_check=True)
                                nc.tensor.matmul(sps[:, w],
                                                 lhsT=kr[off:off + 64, ks],
                                                 rhs=qr[off:off + 64, qsw],
                                                 start=False, stop=(r < 0),
                                                 skip_group_check=True)
                            if r >= 0:
                                # Add the causal tri mask on the PE itself
                                # (identity @ tri) so exp never waits on a
                                # cross-engine DVE hop.
                                nc.tensor.matmul(
                                    sps[:, col0:col0 + 128], lhsT=ident_sb,
                                    rhs=masks_sb, start=False, stop=True,
                                    skip_group_check=True)
                            # Chunk 0's exp writes the accumulator directly
                            # (no copy); later chunks go to a pt tile and
                            # fold in with one elementwise add.
                            if c == 0:
                                pt = ptacc
                            else:
                                pt = apool.tile([128, QB], cdt,
                                                name=f"pt_{h}_{j}_{c}",
                                                tag="pt", bufs=4)
                            nc.scalar.activation(
                                pt[:, w], sps[:, w], Exp,
                                scale=(1.0 / (QS * KS)) if fp8_scores else 1.0)
                            nc.tensor.matmul(
                                ops[:, w], lhsT=vt[c][:, h * DV:(h + 1) * DV],
                                rhs=pt[:, w], start=(c == 0),
                                stop=(c == nch - 1), skip_group_check=True)
                            if c > 0:
                                acc_eng.tensor_add(ptacc[:, w], ptacc[:, w],
                                                   pt[:, w])
                            if c == 1 and pend is not None and len(pend) == 4:
                                ph, pj, pops, pacc = pend
                                recb = norm_early(
                                    ph, pj, calc_dps(ph, pj, pacc))
                                pend = (ph, pj, pops, recb, True)
                            if c == 3 and pend is not None and len(pend) == 5:
                                ph, pj, pops, recb, _ = pend
                                norm_late(ph, pj, pops, recb)
                                pend = None
                        pend = (h, j, ops, ptacc)
                # flush the final unit
                ph, pj, pops, pacc = pend
                recb = norm_early(ph, pj, calc_dps(ph, pj, pacc))
                norm_late(ph, pj, pops, recb)

                # ---- Output projection; PSUM reuses the score slots ----
                for m in range(16):
                    for nb in range(NQB):
                        ncols = slice(nb * QB, (nb + 1) * QB)
                        # Alternate between the (now idle) score and out
                        # accumulator slots for a deeper pipeline.
                        if nb % 2 == 0:
                            if fp8_scores:
                                # score slots are 2-bank pair tiles; use
                                # the first bank of one.
                                ps = psS.tile([128, 2 * QB], f32,
                                              name=f"psw_{m}_{nb}", tag="s",
                                              bufs=2)[:, 0:QB]
                            else:
                                ps = psS.tile([128, QB], f32,
                                              name=f"psw_{m}_{nb}", tag="s")
                        else:
                            ps = psO.tile([128, QB], f32,
                                          name=f"psw_{m}_{nb}", tag="o")
                        for r in range(4):
                            nc.tensor.matmul(
                                ps, lhsT=wo_sb[r][:, m * 128:(m + 1) * 128],
                                rhs=outT[r][:, ncols], start=(r == 0),
                                stop=(r == 3))
                        # Fine-grained staging: each [128,512] block DMAs
                        # out as soon as its copy lands, shortening the tail.
                        st = apool.tile([128, QB], f32, name=f"st_{m}_{nb}",
                                        tag="st", bufs=4)
                        if nb % 2 == 0:
                            nc.scalar.copy(st, ps)
                        else:
                            nc.vector.tensor_copy(st, ps)
                        eng = nc.sync if nb % 2 == 0 else nc.scalar
                        eng.dma_start(
                            outp[(m * NQB + nb) * 128:
                                 (m * NQB + nb + 1) * 128, :], st)

    if split_waits:
        split_multi_waits()
    return nc


def get_program(split_waits=True):
    key = (split_waits, USE_CC, POOL_ACC, FP8_SCORES)
    if key not in _PROGRAM:
        _PROGRAM[key] = _build_program(split_waits)
    return _PROGRAM[key]


def make_core_inputs(x, Wq, Wkv_a, Wkv_b, Wo):
    """Host-side sharding/pre-processing. Returns list of 8 input dicts."""
    scale = 1.0 / math.sqrt(DN + DR)

    inv_freq = 1.0 / (ROPE_THETA ** (np.arange(0, DR, 2, dtype=np.float64) / DR))
    t = np.arange(S, dtype=np.float64)
    freqs = np.outer(t, inv_freq)                      # [S, 32]
    cos32 = np.cos(freqs).T.astype(np.float32)         # [32, S]
    sin32 = np.sin(freqs).T.astype(np.float32)
    cosf = np.tile(cos32, (4, 1)).astype(BF16)         # [128, S]
    sinf = np.tile(np.concatenate([-sin32, sin32], axis=0), (2, 1)).astype(BF16)

    row = np.arange(128)[:, None]
    col = np.arange(128)[None, :]
    masks = np.where(col >= row, 0.0, -1e30).astype(BF16)  # [128, 128]
    ident = np.eye(128, dtype=BF16)
    ones = np.ones([128, 1], dtype=BF16)
    onesf = np.ones([1, 128], dtype=BF16)

    Wq_r = np.asarray(Wq, dtype=np.float32).reshape(D_MODEL, N_HEADS, DN + DR)
    Wb_r = np.asarray(Wkv_b, dtype=np.float32).reshape(R, N_HEADS, DN + DV)
    Wo_f = np.asarray(Wo, dtype=np.float32)
    Wkva_f = np.asarray(Wkv_a, dtype=np.float32)
    x_f = np.asarray(x, dtype=np.float32)

    in_maps = []
    for c in range(NCORES):
        b, g = divmod(c, HP)
        heads = list(range(HP * g, HP * g + HP))
        # chunk-major xT: block (t, k) contiguous [128, 512]
        xTc = np.ascontiguousarray(
            x_f[b].T.reshape(16, 128, NQB, QB).transpose(2, 0, 1, 3)
            .reshape(NQB * 16 * 128, QB)).astype(BF16)
        wq_nope = Wq_r[:, heads, :DN].reshape(D_MODEL, HP * DN)
        wq_rope = Wq_r[:, heads, DN:].reshape(D_MODEL, HP * DR)
        wq_c = (np.concatenate([wq_nope, wq_rope], axis=1) * scale).astype(BF16)
        if USE_CC:
            # c_kv latent shard (128 rows of 512) for this core + rope.
            wkva_c = np.ascontiguousarray(np.concatenate(
                [Wkva_f[:, 128 * g:128 * (g + 1)], Wkva_f[:, R:R + DR]],
                axis=1)).astype(BF16)
        else:
            wkva_c = Wkva_f.astype(BF16)
        wbk_c = np.ascontiguousarray(
            Wb_r[:, heads, :DN].reshape(R, HP * DN)).astype(BF16)
        wbv_c = np.ascontiguousarray(
            Wb_r[:, heads, DN:].reshape(R, HP * DV)).astype(BF16)
        wo_c = np.ascontiguousarray(
            Wo_f[HP * g * DV:(HP * g + HP) * DV, :]).astype(BF16)
        in_maps.append({
            "xT": xTc,
            "wq": np.ascontiguousarray(wq_c),
            "wkva": wkva_c,
            "wkvbk": wbk_c,
            "wkvbv": wbv_c,
            "wo": wo_c,
            "cosf": cosf,
            "sinf": sinf,
            "masks": masks,
            "ident": ident,
            "ones": ones,
            "onesf": onesf,
        })
    return in_maps


def gather_output(results):
    """results: list of 8 dicts with 'outp' block-major partials."""
    out = np.empty((B, S, D_MODEL), dtype=np.float32)
    for b in range(B):
        acc = results[HP * b]["outp"].astype(np.float32).copy()
        for g in range(1, HP):
            acc += results[HP * b + g]["outp"]
        # blocks (m, nb) -> [D_MODEL, S] -> transpose to [S, D_MODEL]
        out[b] = (acc.reshape(16, NQB, 128, QB).transpose(0, 2, 1, 3)
                  .reshape(D_MODEL, S).T)
    return out


def kernel(x, Wq, Wkv_a, Wkv_b, Wo):
    from concourse.bass_utils import run_bass_kernel_spmd

    nc = get_program()
    in_maps = make_core_inputs(x, Wq, Wkv_a, Wkv_b, Wo)
    res = run_bass_kernel_spmd(nc, in_maps, list(range(NCORES)))
    return gather_output(res.results)



# revision 2
# speedup vs baseline: 1.0869x; 1.0869x over previous
"""Multi-Head Latent Attention (MLA) TRN2 Bass kernel.

Sharding: data-parallel over batch (B=2) x tensor-parallel over heads
(16 heads -> 4 per core) = 8 cores. The kv_lora latent path and shared
rope key are computed replicated within each batch group (cross-core
AllGather / Pool-engine offload both trip the chip power throttle and
net out slower -- measured in a previous session); the final output
projection is computed as per-core partials which the host sums.

All on-device dataflow is "transposed" (feature dim on partitions,
sequence on the free dim) so no PE transposes are ever needed:
  qT      = Wq_perm^T @ xT          [768, S]   fp8 DoubleRow matmuls
                                               (x and Wq pre-scaled to
                                               fp8 on host; nope tiles
                                               written to fp8)
  kv_aT   = Wkv_a^T @ xT            [576, S]   bf16 (c_kvT + k_ropeT;
                                               kept bf16 because c_kv
                                               feeds v and fp8 here
                                               costs ~2.5% output err)
  k_nopeT = Wkv_b_k^T @ c_kvT       [512, S]   (written to fp8)
  v       = (c_kvT chunk)^T-matmuls [S, 512]   (natural layout)
  RoPE applied in transposed layout with a partition-swap DMA + 3 DVE ops
  scoresT[s_k, s_q] per (head, q-block of 512) via a single fp8e4
  DoubleRow matmul per chunk (the 192-dim nope+rope contraction packed as
  2x96 partitions, q/k pre-scaled by 64/32 and descaled inside the exp),
  causal masks added on the 4 diagonal chunks by a bf16 PE matmul,
  exp on ACT (no max subtraction; scores are bounded), softmax
  denominators via running elementwise bf16 sums of the exp'd chunks on
  the DVE + a single ones-matmul per (head, q-block), outT accumulated
  in PSUM, normalized by broadcasted reciprocals, then
  partialT = Wo_c^T @ outT, staged per [128,512] block and DMA'd out
  in bf16 (host sums partials in f32).

x and outp use chunk-major DRAM layouts so every DMA is one dense
contiguous block (host packs/unpacks).
"""

import math
import sys

import numpy as np
import ml_dtypes

try:  # concourse ships in the container; fall back to the repo checkout
    import concourse.bass  # noqa: F401
except ImportError:  # pragma: no cover
    for p in ("/opt/trn_rl_repo", "/root/.axon_site/_ro/trn_rl_repo"):
        if p not in sys.path:
            sys.path.insert(0, p)

# Problem constants (hardcoded; harness calls kernel() standalone).
D_MODEL = 2048
N_HEADS = 16
R = 512          # kv lora rank
DN = 128         # d_nope
DR = 64          # d_rope
DV = 128         # d_v
ROPE_THETA = 10000.0
B = 2
S = 2048
HP = 4           # heads per core
QB = 512         # q block size
NKC = S // 128   # 16 k chunks
NQB = S // QB    # 4 q blocks
NCORES = 8

BF16 = ml_dtypes.bfloat16
F8E4 = ml_dtypes.float8_e4m3  # mybir float8e4 (IEEE e4m3, max finite 240)

# fp8 pre-scales. QS/KS: q/k nope+rope tiles feeding the score matmuls
# (descaled inside the exp). SXQ/SWQ: host-side scales for x / Wq feeding
# the fp8 DoubleRow q-projection (descaled in the PSUM->SBUF store).
QS, KS = 64.0, 32.0
SXQ, SWQ = 16.0, 8192.0

_PROGRAM = {}


def _build_program(split_waits=True):
    import concourse.bass as bass
    import concourse.mybir as mybir
    from concourse.tile import TileContext

    def split_multi_waits(max_waits=1):
        """The walrus build in this container rejects instructions with
        more than `max_waits` sync-wait commands. Move excess waits onto
        same-engine NoOps inserted just before the instruction."""
        for f in nc.m.functions:
            for bb in f.blocks:
                out = []
                changed = False
                for inst in bb.instructions:
                    si = getattr(inst, "sync_info", None)
                    ws = list(si.on_wait) if si is not None else []
                    if len(ws) > max_waits:
                        changed = True
                        inst.sync_info = mybir.SyncInfo(
                            on_wait=ws[:max_waits],
                            on_update=list(si.on_update))
                        for w in ws[max_waits:]:
                            n = mybir.InstNoOp(
                                name=nc.get_next_instruction_name(),
                                ins=[], outs=[])
                            n.engine = inst.engine
                            n.sync_info = mybir.SyncInfo(
                                on_wait=[w], on_update=[])
                            out.append(n)
                    out.append(inst)
                if changed:
                    bb.instructions = out

    f32 = mybir.dt.float32
    cdt = mybir.dt.bfloat16
    f8 = mybir.dt.float8e4
    DRmode = mybir.MatmulPerfMode.DoubleRow
    Copy = mybir.ActivationFunctionType.Copy

    nc = bass.Bass()

    # x arrives chunk-major: block (t, k) = xT_logical[128k:128k+128,
    # 512t:512t+512] stored contiguously so every chunk DMA is one dense
    # 128KB transfer instead of 128 separate 1KB rows.
    xT = nc.dram_tensor("xT", [NQB * 16 * 128, QB], cdt, kind="ExternalInput")
    # fp8 copy of x for the q-projection, pair-major for DoubleRow:
    # block (t, p) = [128, 2, 512]: elem (kp, j, col) =
    # SXQ * x[512t+col, 256p+128j+kp] (feature on partitions).
    xT8 = nc.dram_tensor("xT8", [NQB * 8 * 128, 2 * QB], f8,
                         kind="ExternalInput")
    # Wq in fp8, pair-major: block p = [128, 2, 768]: elem (kp, j, m) =
    # SWQ * scale * Wq[256p+128j+kp, m] (m = head-major nope|rope cols).
    wq8 = nc.dram_tensor("wq8", [8 * 128, 2 * HP * (DN + DR)], f8,
                         kind="ExternalInput")
    wkva = nc.dram_tensor("wkva", [D_MODEL, R + DR], cdt, kind="ExternalInput")
    wkvbk = nc.dram_tensor("wkvbk", [R, HP * DN], cdt, kind="ExternalInput")
    wkvbv = nc.dram_tensor("wkvbv", [R, HP * DV], cdt, kind="ExternalInput")
    wo = nc.dram_tensor("wo", [HP * DV, D_MODEL], cdt, kind="ExternalInput")
    cosf = nc.dram_tensor("cosf", [128, S], cdt, kind="ExternalInput")
    sinf = nc.dram_tensor("sinf", [128, S], cdt, kind="ExternalInput")
    masks = nc.dram_tensor("masks", [128, 128], cdt, kind="ExternalInput")
    ident = nc.dram_tensor("ident", [128, 128], cdt, kind="ExternalInput")
    ones = nc.dram_tensor("ones", [128, 1], cdt, kind="ExternalInput")
    onesf = nc.dram_tensor("onesf", [1, 128], cdt, kind="ExternalInput")
    # outp is block-major: block (m, nb) stored contiguously; host unpacks.
    # bf16 partials (host sums in f32) to halve the output DMA drain.
    outp = nc.dram_tensor("outp", [16 * NQB * 128, QB], cdt,
                          kind="ExternalOutput")

    Exp = mybir.ActivationFunctionType.Exp
    Ln = mybir.ActivationFunctionType.Ln

    with TileContext(nc) as tc:
        with (
            tc.tile_pool(name="const", bufs=1) as cpool,
            tc.tile_pool(name="persist", bufs=1) as ppool,
        ):
            cosf_sb = cpool.tile([128, S], cdt, name="cosf_sb")
            sinf_sb = cpool.tile([128, S], cdt, name="sinf_sb")
            masks_sb = cpool.tile([128, 128], cdt, name="masks_sb")
            ident_sb = cpool.tile([128, 128], cdt, name="ident_sb")
            ones_sb = cpool.tile([128, 1], cdt, name="ones_sb")
            onesb_sb = cpool.tile([1, 128], cdt, name="onesb_sb")

            # Persistent activations. q_nope / k_nope tiles live in fp8
            # (written pre-scaled straight from PSUM); rope halves stay
            # bf16 until after the RoPE rotation, then are cast.
            qT8 = [
                ppool.tile([128, S], f8, name=f"qT8_{m}", tag="qT8", bufs=4)
                for m in range(4)
            ]
            qTr = [
                ppool.tile([128, S], cdt, name=f"qTr{m}", tag="qT", bufs=2)
                for m in range(2)
            ]
            qT = qT8 + qTr
            kn8 = [
                ppool.tile([128, S], f8, name=f"kn8_{m}", tag="kn8", bufs=4)
                for m in range(4)
            ]
            qr8 = [
                ppool.tile([128, S], f8, name=f"qr8_{m}", tag="qr8", bufs=2)
                for m in range(2)
            ]
            kr8 = ppool.tile([128, S], f8, name="kr8", tag="kr8", bufs=1)
            # DoubleRow-packed per-head tensors: 192 contraction dims
            # as 2 blocks of 96 partitions (blk0 = dims 0:96,
            # blk1 = dims 96:192 = nope 96:128 + rope 0:64).
            qf8 = [
                ppool.tile([96, NQB, 2, QB], f8, name=f"qf8_{h}", tag="qf8",
                           bufs=4)
                for h in range(HP)
            ]
            kf8 = [
                ppool.tile([96, NKC, 2, 128], f8, name=f"kf8_{h}", tag="kf8",
                           bufs=4)
                for h in range(HP)
            ]
            ck = [
                ppool.tile([128, S], cdt, name=f"ck{m}", tag="cko", bufs=4)
                for m in range(4)
            ]
            kr = ppool.tile([128, S], cdt, name="krope", tag="krope", bufs=1)

            def store_q(m, cols, ps):
                # PSUM -> SBUF store for qT M-tile m, descaling the host
                # fp8 pre-scales (SXQ*SWQ); nope tiles also pick up the
                # QS score pre-scale and go straight to fp8.
                if m < 4:
                    nc.scalar.activation(qT8[m][:, cols], ps, Copy,
                                         scale=QS / (SXQ * SWQ))
                else:
                    nc.scalar.activation(qT[m][:, cols], ps, Copy,
                                         scale=1.0 / (SXQ * SWQ))

            def store_kn(m, cols, ps):
                nc.scalar.activation(kn8[m][:, cols], ps, Copy, scale=KS)

            vt = [
                ppool.tile([128, HP * DV], cdt, name=f"v{i}", tag="v",
                           bufs=NKC)
                for i in range(NKC)
            ]
            # RoPE swap scratch lives in the persistent pool so the kvT
            # weight pool does not WAR-serialize against the rope phase.
            swt = [
                ppool.tile([128, S], cdt, name=f"sw{i}", tag="sw", bufs=3)
                for i in range(3)
            ]

            # kv_b weights persist so their DMAs can issue at startup.
            wbk_sb = [
                ppool.tile([128, HP * DN], cdt, name=f"wbk_sb{r}", tag="wbk",
                           bufs=4)
                for r in range(4)
            ]
            wbv_sb = [
                ppool.tile([128, HP * DV], cdt, name=f"wbv_sb{r}", tag="wbv",
                           bufs=4)
                for r in range(4)
            ]

            # ---- Phase 1: x projections, kv up-projection ----
            with (
                tc.tile_pool(name="wproj", bufs=1) as wpool,
                tc.tile_pool(name="xstream", bufs=1) as xpool,
                tc.tile_pool(name="psA", bufs=8, space="PSUM") as psA,
            ):
                # Weight DMAs on the scalar HWDGE queue; wkva first (the
                # very first PE matmul needs wkva_sb[0]), wq8 interleaved.
                wkva_sb = []
                wq8_sb = []
                for k in range(16):
                    w2 = wpool.tile([128, R + DR], cdt, name=f"wkva_sb{k}",
                                    tag="wkva", bufs=16)
                    nc.scalar.dma_start(w2, wkva[k * 128:(k + 1) * 128, :])
                    wkva_sb.append(w2)
                    if k < 8:
                        w1 = wpool.tile([128, 2, HP * (DN + DR)], f8,
                                        name=f"wq8_sb{k}", tag="wq8", bufs=8)
                        nc.scalar.dma_start(w1, wq8[k * 128:(k + 1) * 128, :])
                        wq8_sb.append(w1)
                for r in range(4):
                    nc.scalar.dma_start(wbk_sb[r],
                                        wkvbk[r * 128:(r + 1) * 128, :])
                    nc.scalar.dma_start(wbv_sb[r],
                                        wkvbv[r * 128:(r + 1) * 128, :])
                nc.scalar.dma_start(cosf_sb, cosf[:, :])
                nc.scalar.dma_start(sinf_sb, sinf[:, :])
                nc.scalar.dma_start(masks_sb, masks[:, :])
                nc.scalar.dma_start(ident_sb, ident[:, :])
                nc.scalar.dma_start(ones_sb, ones[:, :])
                nc.scalar.dma_start(onesb_sb, onesf[:, :])

                for t in range(NQB):
                    tcols = slice(t * QB, (t + 1) * QB)
                    xq = []
                    xq8t = []
                    for k in range(16):
                        xk = xpool.tile([128, QB], cdt, name=f"xq_{t}_{k}",
                                        tag="xq", bufs=20)
                        nc.sync.dma_start(
                            xk,
                            xT[(t * 16 + k) * 128:(t * 16 + k + 1) * 128, :])
                        xq.append(xk)
                        if k % 2 == 1:
                            p = k // 2
                            x8 = xpool.tile([128, 2, QB], f8,
                                            name=f"xq8_{t}_{p}", tag="xq8",
                                            bufs=10)
                            nc.sync.dma_start(
                                x8,
                                xT8[(t * 8 + p) * 128:(t * 8 + p + 1) * 128,
                                    :])
                            xq8t.append(x8)
                    # kv_a first (bf16, DMA-latency friendly at t=0): each
                    # landing x chunk feeds 5 matmuls.
                    ps_k = [
                        psA.tile([128, QB], f32, name=f"psk_{t}_{m}",
                                 tag="ps")
                        for m in range(4)
                    ]
                    ps_r = psA.tile([64, QB], f32, name=f"psr_{t}", tag="ps")
                    for k in range(16):
                        for m in range(4):
                            nc.tensor.matmul(
                                ps_k[m],
                                lhsT=wkva_sb[k][:, m * 128:(m + 1) * 128],
                                rhs=xq[k], start=(k == 0), stop=(k == 15))
                        nc.tensor.matmul(
                            ps_r, lhsT=wkva_sb[k][:, R:R + DR], rhs=xq[k],
                            start=(k == 0), stop=(k == 15))
                    for m in range(4):
                        nc.vector.tensor_copy(ck[m][:, tcols], ps_k[m])
                    nc.scalar.copy(kr[0:64, tcols], ps_r)
                    nc.scalar.copy(kr[64:128, tcols], ps_r)
                    # q projection: fp8 DoubleRow, 2 chunks per matmul.
                    ps_q = [
                        psA.tile([128, QB], f32, name=f"psq_{t}_{m}",
                                 tag="ps")
                        for m in range(6)
                    ]
                    for p in range(8):
                        for m in range(6):
                            nc.tensor.matmul(
                                ps_q[m],
                                lhsT=wq8_sb[p][:, :, m * 128:(m + 1) * 128],
                                rhs=xq8t[p], start=(p == 0), stop=(p == 7),
                                perf_mode=DRmode)
                    for m in range(6):
                        store_q(m, tcols, ps_q[m])

                # ---- kv up-projection (k_nopeT, v); same PSUM pool ----
                for m in range(4):
                    for nb in range(NQB):
                        ncols = slice(nb * QB, (nb + 1) * QB)
                        ps = psA.tile([128, QB], f32, name=f"psn_{m}_{nb}",
                                      tag="ps")
                        for r in range(4):
                            nc.tensor.matmul(
                                ps, lhsT=wbk_sb[r][:, m * 128:(m + 1) * 128],
                                rhs=ck[r][:, ncols], start=(r == 0),
                                stop=(r == 3))
                        store_kn(m, ncols, ps)
                for i in range(NKC):
                    ps = psA.tile([128, HP * DV], f32, name=f"psv_{i}",
                                  tag="ps")
                    for r in range(4):
                        nc.tensor.matmul(
                            ps, lhsT=ck[r][:, i * 128:(i + 1) * 128],
                            rhs=wbv_sb[r], start=(r == 0), stop=(r == 3))
                    nc.vector.tensor_copy(vt[i], ps)

            # ---- Phase 2: RoPE rotation (in place; overlaps on DVE) ----
            # rot = x * cosf + swap32(x) * sinf, where swap32 swaps each
            # 32-row half within every 64-row group (signs folded in sinf).
            for idx, tap in enumerate([qT[4], qT[5], kr]):
                sw = swt[idx]
                for blk in range(4):
                    src = (blk ^ 1) * 32
                    nc.sync.dma_start(
                        sw[blk * 32:(blk + 1) * 32, :],
                        tap[src:src + 32, :])
                nc.vector.tensor_mul(tap, tap, cosf_sb)
                nc.vector.tensor_mul(sw, sw, sinf_sb)
                nc.vector.tensor_add(tap, tap, sw)

            # Cast the roped halves to fp8 and assemble the per-head
            # DoubleRow-packed q/k tensors with partition-moving DMAs.
            nc.scalar.activation(qr8[0], qT[4], Copy, scale=QS)
            nc.scalar.activation(qr8[1], qT[5], Copy, scale=QS)
            nc.scalar.activation(kr8, kr, Copy, scale=KS)
            for h in range(HP):
                off = (h % 2) * 64
                ri = h // 2
                nc.sync.dma_start(qf8[h][0:96, :, 0, :], qT8[h][0:96, :])
                nc.sync.dma_start(qf8[h][0:32, :, 1, :], qT8[h][96:128, :])
                nc.sync.dma_start(qf8[h][32:96, :, 1, :],
                                  qr8[ri][off:off + 64, :])
                nc.scalar.dma_start(kf8[h][0:96, :, 0, :], kn8[h][0:96, :])
                nc.scalar.dma_start(kf8[h][0:32, :, 1, :],
                                    kn8[h][96:128, :])
                nc.scalar.dma_start(kf8[h][32:96, :, 1, :],
                                    kr8[off:off + 64, :])

            # outT tiles reuse the c_kvT slots (same tag, 4 bufs).
            outT = [
                ppool.tile([128, S], cdt, name=f"outT{h}", tag="cko", bufs=4)
                for h in range(HP)
            ]

            # ---- Phase 3: attention + output projection ----
            with (
                tc.tile_pool(name="att", bufs=1) as apool,
                tc.tile_pool(name="psS", bufs=4, space="PSUM") as psS,
                tc.tile_pool(name="psO", bufs=2, space="PSUM") as psO,
                tc.tile_pool(name="psD", bufs=1, space="PSUM") as psD,
                tc.tile_pool(name="psBC", bufs=1, space="PSUM") as psBC,
            ):
                # Wo loads overlap the attention phase on the idle sync
                # queue.
                wo_sb = [
                    apool.tile([128, D_MODEL], cdt, name=f"wo_sb{r}",
                               tag="wo", bufs=4)
                    for r in range(4)
                ]
                for r in range(4):
                    nc.sync.dma_start(wo_sb[r], wo[r * 128:(r + 1) * 128, :])

                def calc_dps(h, j, ptacc):
                    # Single partition-sum matmul over the accumulated
                    # exp'd chunks (deferred off the critical path).
                    dps = psD.tile([1, QB], f32, name=f"dps_{h}_{j}", tag="d")
                    nc.tensor.matmul(dps, lhsT=ones_sb, rhs=ptacc,
                                     start=True, stop=True)
                    return dps

                def norm_early(h, j, dps):
                    # 1/denom as exp(-ln(d)) on the ACT engine. Moving this
                    # (or the bc copy below) to the DVE trips the power
                    # throttle and nets out slower -- measured.
                    rec = apool.tile([1, QB], f32, name=f"rec_{h}_{j}",
                                     tag="rec", bufs=2)
                    nc.scalar.activation(rec, dps, Ln)
                    recb = apool.tile([1, QB], cdt, name=f"recb_{h}_{j}",
                                      tag="recb", bufs=2)
                    nc.scalar.activation(recb, rec, Exp, scale=-1.0)
                    return recb

                def norm_late(h, j, ops, recb):
                    # Broadcast 1/denom across partitions via a K=1 matmul,
                    # then scale the out accumulator into outT.
                    qs = slice(j * QB, (j + 1) * QB)
                    bps = psBC.tile([128, QB], f32, name=f"bps_{h}_{j}",
                                    tag="b")
                    nc.tensor.matmul(bps, lhsT=onesb_sb, rhs=recb,
                                     start=True, stop=True)
                    bc = apool.tile([128, QB], f32, name=f"bc_{h}_{j}",
                                    tag="bc", bufs=2)
                    nc.scalar.copy(bc, bps)
                    nc.vector.tensor_mul(outT[h][:, qs], ops, bc)

                # Deferred normalization pipeline: unit (h,j)'s denominator
                # matmul runs during the NEXT unit's chunk-pair 0 (so the
                # PE never stalls on the accumulation engine), and its
                # normalize runs during the next unit's chunk-pair 1.
                pend = None
                for h in range(HP):
                    for j in range(NQB):
                        qs = slice(j * QB, (j + 1) * QB)
                        ops = psO.tile([128, QB], f32, name=f"ops_{h}_{j}",
                                       tag="o")
                        ptacc = apool.tile([128, QB], cdt,
                                           name=f"pta_{h}_{j}", tag="pta",
                                           bufs=2)
                        nch = 4 * (j + 1)
                        # Chunk PAIRS share one 2-bank PSUM tile so a
                        # single exp covers both chunks -- the ACT
                        # engine is the attention bottleneck and each
                        # activation carries ~280ns fixed overhead.
                        for p in range(nch // 2):
                            cA, cB = 2 * p, 2 * p + 1
                            rA, rB = cA - 4 * j, cB - 4 * j
                            colA = max(0, rA * 128)
                            colB = max(0, rB * 128)
                            wA = slice(colA, QB)
                            wB = slice(colB, QB)
                            pairps = psS.tile(
                                [128, 2 * QB], f32,
                                name=f"sps_{h}_{j}_{p}", tag="s",
                                bufs=2)
                            nc.tensor.matmul(
                                pairps[:, colA:QB],
                                lhsT=kf8[h][:, cA, :, :],
                                rhs=qf8[h][:, j, :, colA:QB],
                                start=True, stop=(rA < 0),
                                perf_mode=DRmode, skip_group_check=True)
                            if rA >= 0:
                                nc.tensor.matmul(
                                    pairps[:, colA:colA + 128],
                                    lhsT=ident_sb, rhs=masks_sb,
                                    start=False, stop=True,
                                    skip_group_check=True)
                            nc.tensor.matmul(
                                pairps[:, QB + colB:2 * QB],
                                lhsT=kf8[h][:, cB, :, :],
                                rhs=qf8[h][:, j, :, colB:QB],
                                start=True, stop=(rB < 0),
                                perf_mode=DRmode, skip_group_check=True)
                            if rB >= 0:
                                nc.tensor.matmul(
                                    pairps[:, QB + colB:QB + colB + 128],
                                    lhsT=ident_sb, rhs=masks_sb,
                                    start=False, stop=True,
                                    skip_group_check=True)
                            ptp = apool.tile([128, 2 * QB], cdt,
                                             name=f"pt_{h}_{j}_{p}",
                                             tag="pt", bufs=2)
                            nc.scalar.activation(
                                ptp[:, colA:2 * QB],
                                pairps[:, colA:2 * QB], Exp,
                                scale=1.0 / (QS * KS))
                            nc.tensor.matmul(
                                ops[:, wA],
                                lhsT=vt[cA][:, h * DV:(h + 1) * DV],
                                rhs=ptp[:, colA:QB], start=(cA == 0),
                                stop=False, skip_group_check=True)
                            nc.tensor.matmul(
                                ops[:, wB],
                                lhsT=vt[cB][:, h * DV:(h + 1) * DV],
                                rhs=ptp[:, QB + colB:2 * QB],
                                start=False, stop=(cB == nch - 1),
                                skip_group_check=True)
                            if cA == 0:
                                nc.vector.tensor_copy(ptacc, ptp[:, 0:QB])
                            else:
                                nc.vector.tensor_add(
                                    ptacc[:, wA], ptacc[:, wA],
                                    ptp[:, colA:QB])
                            nc.vector.tensor_add(
                                ptacc[:, wB], ptacc[:, wB],
                                ptp[:, QB + colB:2 * QB])
                            if p == 0 and pend is not None \
                                    and len(pend) == 4:
                                ph, pj, pops, pacc = pend
                                recb = norm_early(
                                    ph, pj, calc_dps(ph, pj, pacc))
                                pend = (ph, pj, pops, recb, True)
                            if p == 1 and pend is not None \
                                    and len(pend) == 5:
                                ph, pj, pops, recb, _ = pend
                                norm_late(ph, pj, pops, recb)
                                pend = None
                        pend = (h, j, ops, ptacc)
                # flush the final unit
                ph, pj, pops, pacc = pend
                recb = norm_early(ph, pj, calc_dps(ph, pj, pacc))
                norm_late(ph, pj, pops, recb)

                # ---- Output projection; PSUM reuses the score slots ----
                for m in range(16):
                    for nb in range(NQB):
                        ncols = slice(nb * QB, (nb + 1) * QB)
                        # Alternate between the (now idle) score and out
                        # accumulator slots for a deeper pipeline.
                        if nb % 2 == 0:
                            # score slots are 2-bank pair tiles; use
                            # the first bank of one.
                            ps = psS.tile([128, 2 * QB], f32,
                                          name=f"psw_{m}_{nb}", tag="s",
                                          bufs=2)[:, 0:QB]
                        else:
                            ps = psO.tile([128, QB], f32,
                                          name=f"psw_{m}_{nb}", tag="o")
                        for r in range(4):
                            nc.tensor.matmul(
                                ps, lhsT=wo_sb[r][:, m * 128:(m + 1) * 128],
                                rhs=outT[r][:, ncols], start=(r == 0),
                                stop=(r == 3))
                        # Fine-grained staging: each [128,512] block DMAs
                        # out as soon as its copy lands, shortening the
                        # tail.
                        st = apool.tile([128, QB], cdt, name=f"st_{m}_{nb}",
                                        tag="st", bufs=4)
                        if nb % 2 == 0:
                            nc.scalar.copy(st, ps)
                        else:
                            nc.vector.tensor_copy(st, ps)
                        eng = nc.sync if nb % 2 == 0 else nc.scalar
                        eng.dma_start(
                            outp[(m * NQB + nb) * 128:
                                 (m * NQB + nb + 1) * 128, :], st)

    if split_waits:
        split_multi_waits()
    return nc


def get_program(split_waits=True):
    key = (split_waits,)
    if key not in _PROGRAM:
        _PROGRAM[key] = _build_program(split_waits)
    return _PROGRAM[key]


def make_core_inputs(x, Wq, Wkv_a, Wkv_b, Wo):
    """Host-side sharding/pre-processing. Returns list of 8 input dicts."""
    scale = 1.0 / math.sqrt(DN + DR)

    inv_freq = 1.0 / (ROPE_THETA ** (np.arange(0, DR, 2, dtype=np.float64) / DR))
    t = np.arange(S, dtype=np.float64)
    freqs = np.outer(t, inv_freq)                      # [S, 32]
    cos32 = np.cos(freqs).T.astype(np.float32)         # [32, S]
    sin32 = np.sin(freqs).T.astype(np.float32)
    cosf = np.tile(cos32, (4, 1)).astype(BF16)         # [128, S]
    sinf = np.tile(np.concatenate([-sin32, sin32], axis=0), (2, 1)).astype(BF16)

    row = np.arange(128)[:, None]
    col = np.arange(128)[None, :]
    masks = np.where(col >= row, 0.0, -1e30).astype(BF16)  # [128, 128]
    ident = np.eye(128, dtype=BF16)
    ones = np.ones([128, 1], dtype=BF16)
    onesf = np.ones([1, 128], dtype=BF16)

    Wq_r = np.asarray(Wq, dtype=np.float32).reshape(D_MODEL, N_HEADS, DN + DR)
    Wb_r = np.asarray(Wkv_b, dtype=np.float32).reshape(R, N_HEADS, DN + DV)
    Wo_f = np.asarray(Wo, dtype=np.float32)
    Wkva_f = np.asarray(Wkv_a, dtype=np.float32)
    x_f = np.asarray(x, dtype=np.float32)

    in_maps = []
    for c in range(NCORES):
        b, g = divmod(c, HP)
        heads = list(range(HP * g, HP * g + HP))
        # chunk-major xT: block (t, k) contiguous [128, 512]
        xTc = np.ascontiguousarray(
            x_f[b].T.reshape(16, 128, NQB, QB).transpose(2, 0, 1, 3)
            .reshape(NQB * 16 * 128, QB)).astype(BF16)
        # fp8 pair-major x for the DoubleRow q projection.
        xT8c = np.ascontiguousarray(
            (x_f[b].T * SXQ).reshape(8, 2, 128, NQB, QB)
            .transpose(3, 0, 2, 1, 4)
            .reshape(NQB * 8 * 128, 2 * QB)).astype(F8E4)
        wq_nope = Wq_r[:, heads, :DN].reshape(D_MODEL, HP * DN)
        wq_rope = Wq_r[:, heads, DN:].reshape(D_MODEL, HP * DR)
        wq_c = np.concatenate([wq_nope, wq_rope], axis=1) * (scale * SWQ)
        wq8_c = np.ascontiguousarray(
            wq_c.reshape(8, 2, 128, HP * (DN + DR))
            .transpose(0, 2, 1, 3)
            .reshape(8 * 128, 2 * HP * (DN + DR))).astype(F8E4)
        wkva_c = Wkva_f.astype(BF16)
        wbk_c = np.ascontiguousarray(
            Wb_r[:, heads, :DN].reshape(R, HP * DN)).astype(BF16)
        wbv_c = np.ascontiguousarray(
            Wb_r[:, heads, DN:].reshape(R, HP * DV)).astype(BF16)
        wo_c = np.ascontiguousarray(
            Wo_f[HP * g * DV:(HP * g + HP) * DV, :]).astype(BF16)
        in_maps.append({
            "xT": xTc,
            "xT8": xT8c,
            "wq8": wq8_c,
            "wkva": wkva_c,
            "wkvbk": wbk_c,
            "wkvbv": wbv_c,
            "wo": wo_c,
            "cosf": cosf,
            "sinf": sinf,
            "masks": masks,
            "ident": ident,
            "ones": ones,
            "onesf": onesf,
        })
    return in_maps


def gather_output(results):
    """results: list of 8 dicts with 'outp' block-major bf16 partials."""
    out = np.empty((B, S, D_MODEL), dtype=np.float32)
    for b in range(B):
        acc = results[HP * b]["outp"].astype(np.float32)
        for g in range(1, HP):
            acc += results[HP * b + g]["outp"].astype(np.float32)
        # blocks (m, nb) -> [D_MODEL, S] -> transpose to [S, D_MODEL]
        out[b] = (acc.reshape(16, NQB, 128, QB).transpose(0, 2, 1, 3)
                  .reshape(D_MODEL, S).T)
    return out


def kernel(x, Wq, Wkv_a, Wkv_b, Wo):
    from concourse.bass_utils import run_bass_kernel_spmd

    nc = get_program()
    in_maps = make_core_inputs(x, Wq, Wkv_a, Wkv_b, Wo)
    res = run_bass_kernel_spmd(nc, in_maps, list(range(NCORES)))
    return gather_output(res.results)
